# revision 8
# baseline (speedup 1.0000x reference)
"""Trainium2 Bass kernel for nn_MeshLoss.

The reference loss is:
    loss = mean((network_mesh - fem_mesh)^2)
         + 0.1 * sum_{dx,dy,dz} sum_spatial(mean_{B,C}(diff^2))
The chamfer/KNN block in the reference is dead code (its results are unused),
and `pc` does not influence the output, so the kernel computes only the two
reduction terms.

Sharding (8 cores): pred is viewed as 12*32 = 384 (bc, x) planes of [32, 32].
The 12*31 = 372 planes with x < 31 are the regularization bases (padded with
zero planes to 384); a parallel array holds each base's x+1 neighbor plane so
the x-difference is elementwise.  y/z differences are in-plane and expressed
with strided access patterns, so no masking is needed.  network_mesh/fem_mesh
are split into 48 [32, 32] row-planes per core.  Each core receives ONE packed
input blob [48, 4, 32, 32] (base, neighbor, net, fem) — a single input DMA
keeps the kernel at 3 semaphores (DVE + 2 DMA lanes), the CoreV3 tail-drain
sync-wait limit.  All compute runs on the Vector engine for the same reason.
Each core emits per-partition partial sums [48, 4]; the host sums the 8
outputs and applies the 1/N and 0.1/12 weights.
"""

import numpy as np

B, C, X, Y, Z = 4, 3, 32, 32, 32
N_CORES = 8
FEM_TOTAL = B * C * X * Y * Z          # 393216
REG_PLANES = B * C * (X - 1)           # 372 valid base planes
ROWS = 48                              # planes per core (8*48 = 384)

_PROGRAM = None
_HOOK_PATCHED = False


def _fix_drain_waits(bir_json):
    """Walrus in this toolchain rejects instructions with >2 sync commands;
    Tile's kernel-tail drain waits on every proc used (no transitive
    reduction).  This kernel is a single dependency chain ending in the
    output DMA, whose completion implies every earlier wait, so the drain
    only needs that one semaphore."""
    import json

    j = json.loads(bir_json)
    for f in j.get("functions", []):
        last_dma_update = None
        for bb in f.get("blocks", []):
            for i in bb.get("instructions", []):
                if i.get("opcode") == "DMACopy":
                    ups = (i.get("sync_info") or {}).get("on_update") or []
                    if ups:
                        last_dma_update = ups[-1]
        if last_dma_update is None:
            continue
        for bb in f.get("blocks", []):
            for i in bb.get("instructions", []):
                if i.get("opcode") != "Drain":
                    continue
                si = i.get("sync_info") or {}
                waits = si.get("on_wait") or []
                if len(waits) + len(si.get("on_update") or []) <= 2:
                    continue
                keep = [w for w in waits if w.get("id") == last_dma_update.get("id")]
                assert keep, f"tail drain lacks final-DMA wait: {waits}"
                si["on_wait"] = keep
    return json.dumps(j).encode()


def _patch_compile_hook():
    global _HOOK_PATCHED
    if _HOOK_PATCHED:
        return
    import concourse.bass2jax as b2j

    orig = b2j.compile_bir_kernel

    def patched(bir_json, tmpdir, neff_name="file.neff"):
        return orig(_fix_drain_waits(bir_json), tmpdir, neff_name=neff_name)

    b2j.compile_bir_kernel = patched
    _HOOK_PATCHED = True


def _build_program():
    import concourse.bass as bass
    import concourse.mybir as mybir
    from concourse import tile
    from contextlib import ExitStack

    f32 = mybir.dt.float32
    SUB = mybir.AluOpType.subtract
    MULT = mybir.AluOpType.mult

    nc = bass.Bass()
    inp = nc.declare_dram_parameter("inp", [ROWS, 4, Y, Z], f32, isOutput=False)
    out = nc.declare_dram_parameter("out", [ROWS, 4], f32, isOutput=True)

    with tile.TileContext(nc) as tc, ExitStack() as ctx:
        pool = ctx.enter_context(tc.tile_pool(name="main", bufs=1))

        t = pool.tile([ROWS, 4, Y, Z], f32)
        nc.sync.dma_start(out=t[:], in_=inp[:, :, :, :])

        t_pack = pool.tile([ROWS, 4], f32)

        # (diff source, output column); chunk 0 = base plane, 1 = x+1 plane,
        # 2 = net rows, 3 = fem rows
        specs = [
            (t[:, 2:3, :, :], t[:, 3:4, :, :], 0),          # fem MSE
            (t[:, 1:2, 0:31, 0:31], t[:, 0:1, 0:31, 0:31], 1),  # dx
            (t[:, 0:1, 1:32, 0:31], t[:, 0:1, 0:31, 0:31], 2),  # dy
            (t[:, 0:1, 0:31, 1:32], t[:, 0:1, 0:31, 0:31], 3),  # dz
        ]
        for a, b, col in specs:
            shp = list(a.shape)
            t_d = pool.tile(shp, f32, tag=f"d{col}")
            t_sq = pool.tile(shp, f32, tag=f"sq{col}")
            nc.vector.tensor_tensor(out=t_d[:], in0=a, in1=b, op=SUB)
            nc.vector.scalar_tensor_tensor(out=t_sq[:], in0=t_d[:], scalar=1.0,
                                           in1=t_d[:], op0=MULT, op1=MULT,
                                           accum_out=t_pack[0:ROWS, col:col + 1])

        nc.sync.dma_start(out=out[:, :], in_=t_pack[:])

    return nc


def _shard_inputs(network_mesh, fem_mesh, pred):
    predf = np.asarray(pred, dtype=np.float32).reshape(B * C, X, Y, Z)
    pad = N_CORES * ROWS
    base_p = np.zeros((pad, Y, Z), np.float32)
    nxt_p = np.zeros((pad, Y, Z), np.float32)
    base_p[:REG_PLANES] = predf[:, : X - 1].reshape(REG_PLANES, Y, Z)
    nxt_p[:REG_PLANES] = predf[:, 1:].reshape(REG_PLANES, Y, Z)
    netf = np.asarray(network_mesh, dtype=np.float32).reshape(pad, Y, Z)
    femf = np.asarray(fem_mesh, dtype=np.float32).reshape(pad, Y, Z)
    blob = np.stack([base_p, nxt_p, netf, femf], axis=1)  # [384, 4, Y, Z]
    return [
        {"inp": np.ascontiguousarray(blob[ROWS * c : ROWS * (c + 1)])}
        for c in range(N_CORES)
    ]


def run_sharded(network_mesh, fem_mesh, pred, trace=False):
    """Compile+run on 8 cores; returns (loss_scalar, BassKernelResults)."""
    global _PROGRAM
    from concourse.bass_utils import run_bass_kernel_spmd

    _patch_compile_hook()
    if _PROGRAM is None:
        _PROGRAM = _build_program()
    in_maps = _shard_inputs(network_mesh, fem_mesh, pred)
    res = run_bass_kernel_spmd(_PROGRAM, in_maps, list(range(N_CORES)), trace=trace)
    fem_sum = 0.0
    reg_sum = 0.0
    for c in range(N_CORES):
        o = np.asarray(res.results[c]["out"], dtype=np.float64)
        fem_sum += o[:, 0].sum()
        reg_sum += o[:, 1:4].sum()
    loss = fem_sum / FEM_TOTAL + 0.1 * (reg_sum / (B * C))
    return np.asarray(loss, dtype=np.float32), res


def kernel(network_mesh, pc, fem_mesh, pred):
    loss, _ = run_sharded(network_mesh, fem_mesh, pred, trace=False)
    return loss


# revision 9
# speedup vs baseline: 1.4427x; 1.4427x over previous
"""Trainium2 Bass kernel for nn_MeshLoss.

The reference loss is:
    loss = mean((network_mesh - fem_mesh)^2)
         + 0.1 * sum_{dx,dy,dz} sum_spatial(mean_{B,C}(diff^2))
The chamfer/KNN block in the reference is dead code (its results are unused),
and `pc` does not influence the output, so the kernel computes only the two
reduction terms.

Sharding (8 cores): pred is viewed as 12*32 = 384 (bc, x) planes of [32, 32];
the 12*31 = 372 planes with x < 31 are regularization bases, 46-47 per core.
On the host each (plane, y<31) pair becomes a 3-row unit [base row, y+1 row,
x+1-plane row]; a core's 48*31 units (zero-padded to 1536) are laid out as
[128, 12, 3, 32], so every difference is an elementwise op over all 128
partitions with the y/z "::-1" bounds expressed as strided access patterns —
no masking, no invalid contributions.  network_mesh/fem_mesh are split into 8
chunks reshaped [128, 384] and packed side by side as [128, 768].

All compute runs on the Vector engine and there are only three DMAs; walrus
in this toolchain rejects instructions with more than 2 sync commands, so the
kernel must stay a single dependency chain (see _fix_drain_waits).
Each core emits per-partition partial sums [128, 4]; the host sums the 8
outputs and applies the 1/N and 0.1/12 weights.
"""

import numpy as np

B, C, X, Y, Z = 4, 3, 32, 32, 32
N_CORES = 8
FEM_TOTAL = B * C * X * Y * Z          # 393216
REG_PLANES = B * C * (X - 1)           # 372 valid base planes
PLANES_PC = 48                         # plane slots per core (8*48 = 384)
UNITS_PC = PLANES_PC * (Y - 1)         # 1488 (plane, y) units per core
KU = 12                                # units per partition (128*12 = 1536)
FEM_P, FEM_F = 128, FEM_TOTAL // N_CORES // 128   # [128, 384] per core

_PROGRAM = None
_HOOK_PATCHED = False


def _fix_drain_waits(bir_json):
    """Walrus in this toolchain rejects instructions with >2 sync commands;
    Tile's kernel-tail drain waits on every proc used (no transitive
    reduction).  This kernel is a single dependency chain ending in the
    output DMA, whose completion implies every earlier wait, so the drain
    only needs that one semaphore."""
    import json

    j = json.loads(bir_json)
    for f in j.get("functions", []):
        last_dma_update = None
        for bb in f.get("blocks", []):
            for i in bb.get("instructions", []):
                if i.get("opcode") == "DMACopy":
                    ups = (i.get("sync_info") or {}).get("on_update") or []
                    if ups:
                        last_dma_update = ups[-1]
        if last_dma_update is None:
            continue
        for bb in f.get("blocks", []):
            for i in bb.get("instructions", []):
                if i.get("opcode") != "Drain":
                    continue
                si = i.get("sync_info") or {}
                waits = si.get("on_wait") or []
                if len(waits) + len(si.get("on_update") or []) <= 2:
                    continue
                keep = [w for w in waits if w.get("id") == last_dma_update.get("id")]
                assert keep, f"tail drain lacks final-DMA wait: {waits}"
                si["on_wait"] = keep
    return json.dumps(j).encode()


def _patch_compile_hook():
    global _HOOK_PATCHED
    if _HOOK_PATCHED:
        return
    import concourse.bass2jax as b2j

    orig = b2j.compile_bir_kernel

    def patched(bir_json, tmpdir, neff_name="file.neff"):
        return orig(_fix_drain_waits(bir_json), tmpdir, neff_name=neff_name)

    b2j.compile_bir_kernel = patched
    _HOOK_PATCHED = True


def _build_program():
    import concourse.bass as bass
    import concourse.mybir as mybir
    from concourse import tile
    from contextlib import ExitStack

    f32 = mybir.dt.float32
    SUB = mybir.AluOpType.subtract
    MULT = mybir.AluOpType.mult

    nc = bass.Bass()
    nf = nc.declare_dram_parameter("nf", [FEM_P, 2 * FEM_F], f32, isOutput=False)
    u = nc.declare_dram_parameter("u", [128, KU, 3, Z], f32, isOutput=False)
    out = nc.declare_dram_parameter("out", [128, 4], f32, isOutput=True)

    with tile.TileContext(nc) as tc, ExitStack() as ctx:
        pool = ctx.enter_context(tc.tile_pool(name="main", bufs=1))

        t_nf = pool.tile([FEM_P, 2 * FEM_F], f32)
        t_u = pool.tile([128, KU, 3, Z], f32)
        nc.sync.dma_start(out=t_nf[:], in_=nf[:, :])
        nc.sync.dma_start(out=t_u[:], in_=u[:, :, :, :])

        t_pack = pool.tile([128, 4], f32)

        # fem MSE partial: (net - fem)^2 row sums -> pack[:, 0]
        t_fd = pool.tile([FEM_P, FEM_F], f32)
        t_fsq = pool.tile([FEM_P, FEM_F], f32)
        nc.vector.tensor_tensor(out=t_fd[:], in0=t_nf[:, 0:FEM_F],
                                in1=t_nf[:, FEM_F:2 * FEM_F], op=SUB)
        nc.vector.scalar_tensor_tensor(out=t_fsq[:], in0=t_fd[:], scalar=1.0,
                                       in1=t_fd[:], op0=MULT, op1=MULT,
                                       accum_out=t_pack[0:FEM_P, 0:1])

        # regularization partials: unit row 0 = base, 1 = y+1 row, 2 = x+1 row
        base = t_u[:, :, 0, 0:31]
        shifts = [
            (t_u[:, :, 2, 0:31], 1),   # dx
            (t_u[:, :, 1, 0:31], 2),   # dy
            (t_u[:, :, 0, 1:32], 3),   # dz
        ]
        for shifted, col in shifts:
            t_d = pool.tile([128, KU, 31], f32, tag=f"d{col}")
            t_sq = pool.tile([128, KU, 31], f32, tag=f"sq{col}")
            nc.vector.tensor_tensor(out=t_d[:], in0=shifted, in1=base, op=SUB)
            nc.vector.scalar_tensor_tensor(out=t_sq[:], in0=t_d[:], scalar=1.0,
                                           in1=t_d[:], op0=MULT, op1=MULT,
                                           accum_out=t_pack[0:128, col:col + 1])

        nc.sync.dma_start(out=out[:, :], in_=t_pack[:])

    return nc


def _shard_inputs(network_mesh, fem_mesh, pred):
    predf = np.asarray(pred, dtype=np.float32).reshape(B * C, X, Y, Z)
    pad = N_CORES * PLANES_PC
    base_p = np.zeros((pad, Y, Z), np.float32)
    nxt_p = np.zeros((pad, Y, Z), np.float32)
    base_p[:REG_PLANES] = predf[:, : X - 1].reshape(REG_PLANES, Y, Z)
    nxt_p[:REG_PLANES] = predf[:, 1:].reshape(REG_PLANES, Y, Z)
    # [384, 31, 3, 32]: per (plane, y): base row, y+1 row, x+1-plane row
    u_all = np.stack(
        [base_p[:, : Y - 1], base_p[:, 1:], nxt_p[:, : Y - 1]], axis=2
    )
    netf = np.asarray(network_mesh, dtype=np.float32).reshape(N_CORES, FEM_P, FEM_F)
    femf = np.asarray(fem_mesh, dtype=np.float32).reshape(N_CORES, FEM_P, FEM_F)
    nf = np.concatenate([netf, femf], axis=2)  # [N_CORES, 128, 768]
    maps = []
    for c in range(N_CORES):
        uc = u_all[PLANES_PC * c : PLANES_PC * (c + 1)].reshape(UNITS_PC, 3, Z)
        up = np.zeros((128 * KU, 3, Z), np.float32)
        up[:UNITS_PC] = uc
        maps.append({
            "nf": np.ascontiguousarray(nf[c]),
            "u": up.reshape(128, KU, 3, Z),
        })
    return maps


def run_sharded(network_mesh, fem_mesh, pred, trace=False):
    """Compile+run on 8 cores; returns (loss_scalar, BassKernelResults)."""
    global _PROGRAM
    from concourse.bass_utils import run_bass_kernel_spmd

    _patch_compile_hook()
    if _PROGRAM is None:
        _PROGRAM = _build_program()
    in_maps = _shard_inputs(network_mesh, fem_mesh, pred)
    res = run_bass_kernel_spmd(_PROGRAM, in_maps, list(range(N_CORES)), trace=trace)
    fem_sum = 0.0
    reg_sum = 0.0
    for c in range(N_CORES):
        o = np.asarray(res.results[c]["out"], dtype=np.float64)
        fem_sum += o[:, 0].sum()
        reg_sum += o[:, 1:4].sum()
    loss = fem_sum / FEM_TOTAL + 0.1 * (reg_sum / (B * C))
    return np.asarray(loss, dtype=np.float32), res


def kernel(network_mesh, pc, fem_mesh, pred):
    loss, _ = run_sharded(network_mesh, fem_mesh, pred, trace=False)
    return loss


# revision 10
# speedup vs baseline: 1.5355x; 1.0643x over previous
"""Trainium2 Bass kernel for nn_MeshLoss.

The reference loss is:
    loss = mean((network_mesh - fem_mesh)^2)
         + 0.1 * sum_{dx,dy,dz} sum_spatial(mean_{B,C}(diff^2))
The chamfer/KNN block in the reference is dead code (its results are unused),
and `pc` does not influence the output, so the kernel computes only the two
reduction terms.

Sharding (8 cores): pred is viewed as 12*32 = 384 (bc, x) planes of [32, 32];
the 12*31 = 372 planes with x < 31 are regularization bases, 46-47 per core.
On the host each (plane, y<31) pair becomes a 3-row unit [base row, y+1 row,
x+1-plane row]; a core's 48*31 units (zero-padded to 1536) are laid out as
[128, 12, 3, 32], so every difference is an elementwise op over all 128
partitions with the y/z "::-1" bounds expressed as strided access patterns —
no masking, no invalid contributions.  network_mesh/fem_mesh are split into 8
chunks reshaped [128, 384] and packed side by side as [128, 768].

All compute runs on the Vector engine and there are only three DMAs; walrus
in this toolchain rejects instructions with more than 2 sync commands, so the
kernel must stay a single dependency chain (see _fix_drain_waits).
Each core emits per-partition partial sums [128, 4]; the host sums the 8
outputs and applies the 1/N and 0.1/12 weights.
"""

import numpy as np

B, C, X, Y, Z = 4, 3, 32, 32, 32
N_CORES = 8
FEM_TOTAL = B * C * X * Y * Z          # 393216
REG_PLANES = B * C * (X - 1)           # 372 valid base planes
PLANES_PC = 48                         # plane slots per core (8*48 = 384)
UNITS_PC = PLANES_PC * (Y - 1)         # 1488 (plane, y) units per core
KU = 12                                # units per partition (128*12 = 1536)
FEM_P, FEM_F = 128, FEM_TOTAL // N_CORES // 128   # [128, 384] per core

_PROGRAM = None
_HOOK_PATCHED = False


def _fix_drain_waits(bir_json):
    """Walrus in this toolchain rejects instructions with >2 sync commands;
    Tile's kernel-tail drain waits on every proc used (no transitive
    reduction).  This kernel is a single dependency chain ending in the
    output DMA, whose completion implies every earlier wait, so the drain
    only needs that one semaphore."""
    import json

    j = json.loads(bir_json)
    for f in j.get("functions", []):
        last_dma_update = None
        for bb in f.get("blocks", []):
            for i in bb.get("instructions", []):
                if i.get("opcode") == "DMACopy":
                    ups = (i.get("sync_info") or {}).get("on_update") or []
                    if ups:
                        last_dma_update = ups[-1]
        if last_dma_update is None:
            continue
        for bb in f.get("blocks", []):
            for i in bb.get("instructions", []):
                if i.get("opcode") != "Drain":
                    continue
                si = i.get("sync_info") or {}
                waits = si.get("on_wait") or []
                if len(waits) + len(si.get("on_update") or []) <= 2:
                    continue
                keep = [w for w in waits if w.get("id") == last_dma_update.get("id")]
                assert keep, f"tail drain lacks final-DMA wait: {waits}"
                si["on_wait"] = keep
    return json.dumps(j).encode()


def _hoist_input_dmas(bir_json, input_names=("nf", "u")):
    """Move the input-load DMA triggers to the head of the first block so the
    HBM->SBUF transfers overlap the ~7.5us framework preamble instead of
    starting after it.  The triggers have no waits, their DMAHW semaphore
    updates don't interact with the barrier semaphores, and consumers keep
    their explicit waits, so ordering stays sound."""
    import json

    j = json.loads(bir_json)
    for f in j.get("functions", []):
        blocks = f.get("blocks", [])
        if not blocks:
            continue
        hoisted = []
        for bb in blocks:
            insts = bb.get("instructions", [])
            keep = []
            for i in insts:
                ins0 = (i.get("ins") or [{}])[0]
                if (i.get("opcode") == "DMACopy"
                        and not (i.get("sync_info") or {}).get("on_wait")
                        and ins0.get("memref") in input_names):
                    hoisted.append(i)
                else:
                    keep.append(i)
            bb["instructions"] = keep
        blocks[0]["instructions"] = hoisted + blocks[0]["instructions"]
    return json.dumps(j).encode()


def _patch_compile_hook():
    global _HOOK_PATCHED
    if _HOOK_PATCHED:
        return
    import concourse.bass2jax as b2j

    orig = b2j.compile_bir_kernel

    def patched(bir_json, tmpdir, neff_name="file.neff"):
        return orig(_hoist_input_dmas(_fix_drain_waits(bir_json)),
                    tmpdir, neff_name=neff_name)

    b2j.compile_bir_kernel = patched
    _HOOK_PATCHED = True


def _build_program():
    import concourse.bass as bass
    import concourse.mybir as mybir
    from concourse import tile
    from contextlib import ExitStack

    f32 = mybir.dt.float32
    SUB = mybir.AluOpType.subtract
    MULT = mybir.AluOpType.mult

    nc = bass.Bass()
    nf = nc.declare_dram_parameter("nf", [FEM_P, 2 * FEM_F], f32, isOutput=False)
    u = nc.declare_dram_parameter("u", [128, KU, 3, Z], f32, isOutput=False)
    out = nc.declare_dram_parameter("out", [128, 4], f32, isOutput=True)

    with tile.TileContext(nc) as tc, ExitStack() as ctx:
        pool = ctx.enter_context(tc.tile_pool(name="main", bufs=1))

        t_nf = pool.tile([FEM_P, 2 * FEM_F], f32)
        t_u = pool.tile([128, KU, 3, Z], f32)
        nc.sync.dma_start(out=t_nf[:], in_=nf[:, :])
        nc.sync.dma_start(out=t_u[:], in_=u[:, :, :, :])

        t_pack = pool.tile([128, 4], f32)

        # fem MSE partial: (net - fem)^2 row sums -> pack[:, 0]
        t_fd = pool.tile([FEM_P, FEM_F], f32)
        t_fsq = pool.tile([FEM_P, FEM_F], f32)
        nc.vector.tensor_tensor(out=t_fd[:], in0=t_nf[:, 0:FEM_F],
                                in1=t_nf[:, FEM_F:2 * FEM_F], op=SUB)
        nc.vector.scalar_tensor_tensor(out=t_fsq[:], in0=t_fd[:], scalar=1.0,
                                       in1=t_fd[:], op0=MULT, op1=MULT,
                                       accum_out=t_pack[0:FEM_P, 0:1])

        # regularization partials: unit row 0 = base, 1 = y+1 row, 2 = x+1 row
        base = t_u[:, :, 0, 0:31]
        shifts = [
            (t_u[:, :, 2, 0:31], 1),   # dx
            (t_u[:, :, 1, 0:31], 2),   # dy
            (t_u[:, :, 0, 1:32], 3),   # dz
        ]
        for shifted, col in shifts:
            t_d = pool.tile([128, KU, 31], f32, tag=f"d{col}")
            t_sq = pool.tile([128, KU, 31], f32, tag=f"sq{col}")
            nc.vector.tensor_tensor(out=t_d[:], in0=shifted, in1=base, op=SUB)
            nc.vector.scalar_tensor_tensor(out=t_sq[:], in0=t_d[:], scalar=1.0,
                                           in1=t_d[:], op0=MULT, op1=MULT,
                                           accum_out=t_pack[0:128, col:col + 1])

        nc.sync.dma_start(out=out[:, :], in_=t_pack[:])

    return nc


def _shard_inputs(network_mesh, fem_mesh, pred):
    predf = np.asarray(pred, dtype=np.float32).reshape(B * C, X, Y, Z)
    pad = N_CORES * PLANES_PC
    base_p = np.zeros((pad, Y, Z), np.float32)
    nxt_p = np.zeros((pad, Y, Z), np.float32)
    base_p[:REG_PLANES] = predf[:, : X - 1].reshape(REG_PLANES, Y, Z)
    nxt_p[:REG_PLANES] = predf[:, 1:].reshape(REG_PLANES, Y, Z)
    # [384, 31, 3, 32]: per (plane, y): base row, y+1 row, x+1-plane row
    u_all = np.stack(
        [base_p[:, : Y - 1], base_p[:, 1:], nxt_p[:, : Y - 1]], axis=2
    )
    netf = np.asarray(network_mesh, dtype=np.float32).reshape(N_CORES, FEM_P, FEM_F)
    femf = np.asarray(fem_mesh, dtype=np.float32).reshape(N_CORES, FEM_P, FEM_F)
    nf = np.concatenate([netf, femf], axis=2)  # [N_CORES, 128, 768]
    maps = []
    for c in range(N_CORES):
        uc = u_all[PLANES_PC * c : PLANES_PC * (c + 1)].reshape(UNITS_PC, 3, Z)
        up = np.zeros((128 * KU, 3, Z), np.float32)
        up[:UNITS_PC] = uc
        maps.append({
            "nf": np.ascontiguousarray(nf[c]),
            "u": up.reshape(128, KU, 3, Z),
        })
    return maps


def run_sharded(network_mesh, fem_mesh, pred, trace=False):
    """Compile+run on 8 cores; returns (loss_scalar, BassKernelResults)."""
    global _PROGRAM
    from concourse.bass_utils import run_bass_kernel_spmd

    _patch_compile_hook()
    if _PROGRAM is None:
        _PROGRAM = _build_program()
    in_maps = _shard_inputs(network_mesh, fem_mesh, pred)
    res = run_bass_kernel_spmd(_PROGRAM, in_maps, list(range(N_CORES)), trace=trace)
    fem_sum = 0.0
    reg_sum = 0.0
    for c in range(N_CORES):
        o = np.asarray(res.results[c]["out"], dtype=np.float64)
        fem_sum += o[:, 0].sum()
        reg_sum += o[:, 1:4].sum()
    loss = fem_sum / FEM_TOTAL + 0.1 * (reg_sum / (B * C))
    return np.asarray(loss, dtype=np.float32), res


def kernel(network_mesh, pc, fem_mesh, pred):
    loss, _ = run_sharded(network_mesh, fem_mesh, pred, trace=False)
    return loss


# revision 12
# speedup vs baseline: 1.5488x; 1.0087x over previous
"""Trainium2 Bass kernel for nn_MeshLoss.

The reference loss is:
    loss = mean((network_mesh - fem_mesh)^2)
         + 0.1 * sum_{dx,dy,dz} sum_spatial(mean_{B,C}(diff^2))
The chamfer/KNN block in the reference is dead code (its results are unused),
and `pc` does not influence the output, so the kernel computes only the two
reduction terms.

Sharding (8 cores): pred is viewed as 12*32 = 384 (bc, x) planes of [32, 32];
the 12*31 = 372 planes with x < 31 are regularization bases, 46-47 per core.
On the host each (plane, y<31) pair becomes a 3-row unit [base row, y+1 row,
x+1-plane row]; a core's 48*31 units (zero-padded to 1536) are laid out as
[128, 12, 3, 32], so every difference is an elementwise op over all 128
partitions with the y/z "::-1" bounds expressed as strided access patterns —
no masking, no invalid contributions.  network_mesh/fem_mesh are split into 8
chunks reshaped [128, 384] and packed side by side as [128, 768].

All compute runs on the Vector engine and there are only three DMAs; walrus
in this toolchain rejects instructions with more than 2 sync commands, so the
kernel must stay a single dependency chain (see _fix_drain_waits).
Each core emits per-partition partial sums [128, 4]; the host sums the 8
outputs and applies the 1/N and 0.1/12 weights.
"""

import numpy as np

B, C, X, Y, Z = 4, 3, 32, 32, 32
N_CORES = 8
FEM_TOTAL = B * C * X * Y * Z          # 393216
REG_PLANES = B * C * (X - 1)           # 372 valid base planes
PLANES_PC = 48                         # plane slots per core (8*48 = 384)
UNITS_PC = PLANES_PC * (Y - 1)         # 1488 (plane, y) units per core
KU = 12                                # units per partition (128*12 = 1536)
FEM_P, FEM_F = 128, FEM_TOTAL // N_CORES // 128   # [128, 384] per core

_PROGRAM = None
_HOOK_PATCHED = False
# Bump whenever the BIR post-edit logic changes: the neuron compile cache
# keys on the HLO (which embeds the *unpatched* BIR), so a patch-logic change
# must perturb the program to force a recompile.
_BIR_REV = 3


def _fix_drain_waits(bir_json):
    """Walrus in this toolchain rejects instructions with >2 sync commands;
    Tile's kernel-tail drain waits on every proc used (no transitive
    reduction).  This kernel is a single dependency chain ending in the
    output DMA, whose completion implies every earlier wait, so the drain
    only needs that one semaphore."""
    import json

    j = json.loads(bir_json)
    for f in j.get("functions", []):
        last_dma_update = None
        for bb in f.get("blocks", []):
            for i in bb.get("instructions", []):
                if i.get("opcode") == "DMACopy":
                    ups = (i.get("sync_info") or {}).get("on_update") or []
                    if ups:
                        last_dma_update = ups[-1]
        if last_dma_update is None:
            continue
        for bb in f.get("blocks", []):
            for i in bb.get("instructions", []):
                if i.get("opcode") != "Drain":
                    continue
                si = i.get("sync_info") or {}
                waits = si.get("on_wait") or []
                if len(waits) + len(si.get("on_update") or []) <= 2:
                    continue
                keep = [w for w in waits if w.get("id") == last_dma_update.get("id")]
                assert keep, f"tail drain lacks final-DMA wait: {waits}"
                si["on_wait"] = keep
    return json.dumps(j).encode()


def _hoist_input_dmas(bir_json, input_names=("nf", "u")):
    """Move the input-load DMA triggers to the head of the first block so the
    HBM->SBUF transfers overlap the ~7.5us framework preamble instead of
    starting after it.  The triggers have no waits, their DMAHW semaphore
    updates don't interact with the barrier semaphores, and consumers keep
    their explicit waits, so ordering stays sound."""
    import json

    j = json.loads(bir_json)
    for f in j.get("functions", []):
        blocks = f.get("blocks", [])
        if not blocks:
            continue
        hoisted = []
        for bb in blocks:
            insts = bb.get("instructions", [])
            keep = []
            for i in insts:
                ins0 = (i.get("ins") or [{}])[0]
                if (i.get("opcode") == "DMACopy"
                        and not (i.get("sync_info") or {}).get("on_wait")
                        and ins0.get("memref") in input_names):
                    hoisted.append(i)
                else:
                    keep.append(i)
            bb["instructions"] = keep
        blocks[0]["instructions"] = hoisted + blocks[0]["instructions"]
    return json.dumps(j).encode()


def _patch_compile_hook():
    global _HOOK_PATCHED
    if _HOOK_PATCHED:
        return
    import concourse.bass2jax as b2j

    orig = b2j.compile_bir_kernel

    def patched(bir_json, tmpdir, neff_name="file.neff"):
        return orig(_hoist_input_dmas(_fix_drain_waits(bir_json)),
                    tmpdir, neff_name=neff_name)

    b2j.compile_bir_kernel = patched
    _HOOK_PATCHED = True


def _build_program():
    import concourse.bass as bass
    import concourse.mybir as mybir
    from concourse import tile
    from contextlib import ExitStack

    f32 = mybir.dt.float32
    SUB = mybir.AluOpType.subtract
    MULT = mybir.AluOpType.mult

    nc = bass.Bass()
    nc.dram_tensor(f"patchrev{_BIR_REV}", [1, 1], f32)
    nf = nc.declare_dram_parameter("nf", [FEM_P, 2 * FEM_F], f32, isOutput=False)
    u = nc.declare_dram_parameter("u", [128, KU, 3, Z], f32, isOutput=False)
    out = nc.declare_dram_parameter("out", [128, 4], f32, isOutput=True)

    with tile.TileContext(nc) as tc, ExitStack() as ctx:
        pool = ctx.enter_context(tc.tile_pool(name="main", bufs=1))

        t_nf = pool.tile([FEM_P, 2 * FEM_F], f32)
        t_u = pool.tile([128, KU, 3, Z], f32)
        nc.sync.dma_start(out=t_nf[:], in_=nf[:, :])
        nc.sync.dma_start(out=t_u[:], in_=u[:, :, :, :])

        t_pack = pool.tile([128, 4], f32)

        # fem MSE partial: (net - fem)^2 row sums -> pack[:, 0]
        t_fd = pool.tile([FEM_P, FEM_F], f32)
        t_fsq = pool.tile([FEM_P, FEM_F], f32)
        nc.vector.tensor_tensor(out=t_fd[:], in0=t_nf[:, 0:FEM_F],
                                in1=t_nf[:, FEM_F:2 * FEM_F], op=SUB)
        nc.vector.scalar_tensor_tensor(out=t_fsq[:], in0=t_fd[:], scalar=1.0,
                                       in1=t_fd[:], op0=MULT, op1=MULT,
                                       accum_out=t_pack[0:FEM_P, 0:1])

        # regularization partials: unit row 0 = base, 1 = y+1 row, 2 = x+1 row
        base = t_u[:, :, 0, 0:31]
        shifts = [
            (t_u[:, :, 2, 0:31], 1),   # dx
            (t_u[:, :, 1, 0:31], 2),   # dy
            (t_u[:, :, 0, 1:32], 3),   # dz
        ]
        for shifted, col in shifts:
            t_d = pool.tile([128, KU, 31], f32, tag=f"d{col}")
            t_sq = pool.tile([128, KU, 31], f32, tag=f"sq{col}")
            nc.vector.tensor_tensor(out=t_d[:], in0=shifted, in1=base, op=SUB)
            nc.vector.scalar_tensor_tensor(out=t_sq[:], in0=t_d[:], scalar=1.0,
                                           in1=t_d[:], op0=MULT, op1=MULT,
                                           accum_out=t_pack[0:128, col:col + 1])

        nc.sync.dma_start(out=out[:, :], in_=t_pack[:])

    return nc


def _shard_inputs(network_mesh, fem_mesh, pred):
    predf = np.asarray(pred, dtype=np.float32).reshape(B * C, X, Y, Z)
    pad = N_CORES * PLANES_PC
    base_p = np.zeros((pad, Y, Z), np.float32)
    nxt_p = np.zeros((pad, Y, Z), np.float32)
    base_p[:REG_PLANES] = predf[:, : X - 1].reshape(REG_PLANES, Y, Z)
    nxt_p[:REG_PLANES] = predf[:, 1:].reshape(REG_PLANES, Y, Z)
    # [384, 31, 3, 32]: per (plane, y): base row, y+1 row, x+1-plane row
    u_all = np.stack(
        [base_p[:, : Y - 1], base_p[:, 1:], nxt_p[:, : Y - 1]], axis=2
    )
    netf = np.asarray(network_mesh, dtype=np.float32).reshape(N_CORES, FEM_P, FEM_F)
    femf = np.asarray(fem_mesh, dtype=np.float32).reshape(N_CORES, FEM_P, FEM_F)
    nf = np.concatenate([netf, femf], axis=2)  # [N_CORES, 128, 768]
    maps = []
    for c in range(N_CORES):
        uc = u_all[PLANES_PC * c : PLANES_PC * (c + 1)].reshape(UNITS_PC, 3, Z)
        up = np.zeros((128 * KU, 3, Z), np.float32)
        up[:UNITS_PC] = uc
        maps.append({
            "nf": np.ascontiguousarray(nf[c]),
            "u": up.reshape(128, KU, 3, Z),
        })
    return maps


def run_sharded(network_mesh, fem_mesh, pred, trace=False):
    """Compile+run on 8 cores; returns (loss_scalar, BassKernelResults)."""
    global _PROGRAM
    from concourse.bass_utils import run_bass_kernel_spmd

    _patch_compile_hook()
    if _PROGRAM is None:
        _PROGRAM = _build_program()
    in_maps = _shard_inputs(network_mesh, fem_mesh, pred)
    res = run_bass_kernel_spmd(_PROGRAM, in_maps, list(range(N_CORES)), trace=trace)
    fem_sum = 0.0
    reg_sum = 0.0
    for c in range(N_CORES):
        o = np.asarray(res.results[c]["out"], dtype=np.float64)
        fem_sum += o[:, 0].sum()
        reg_sum += o[:, 1:4].sum()
    loss = fem_sum / FEM_TOTAL + 0.1 * (reg_sum / (B * C))
    return np.asarray(loss, dtype=np.float32), res


def kernel(network_mesh, pc, fem_mesh, pred):
    loss, _ = run_sharded(network_mesh, fem_mesh, pred, trace=False)
    return loss


# revision 14
# speedup vs baseline: 1.5561x; 1.0047x over previous
"""Trainium2 Bass kernel for nn_MeshLoss.

The reference loss is:
    loss = mean((network_mesh - fem_mesh)^2)
         + 0.1 * sum_{dx,dy,dz} sum_spatial(mean_{B,C}(diff^2))
The chamfer/KNN block in the reference is dead code (its results are unused),
and `pc` does not influence the output, so the kernel computes only the two
reduction terms.

Sharding (8 cores): pred is viewed as 12*32 = 384 (bc, x) planes of [32, 32];
the 12*31 = 372 planes with x < 31 are regularization bases, 46-47 per core.
On the host each (plane, y<31) pair becomes a 3-row unit [base row, y+1 row,
x+1-plane row]; a core's 48*31 units (zero-padded to 1536) are laid out as
[128, 12, 3, 32], so every difference is an elementwise op over all 128
partitions with the y/z "::-1" bounds expressed as strided access patterns —
no masking, no invalid contributions.  network_mesh/fem_mesh are split into 8
chunks reshaped [128, 384] and packed side by side as [128, 768].

All compute runs on the Vector engine and there are only three DMAs; walrus
in this toolchain rejects instructions with more than 2 sync commands, so the
kernel must stay a single dependency chain (see _fix_drain_waits).
Each core emits per-partition partial sums [128, 4]; the host sums the 8
outputs and applies the 1/N and 0.1/12 weights.
"""

import numpy as np

B, C, X, Y, Z = 4, 3, 32, 32, 32
N_CORES = 8
FEM_TOTAL = B * C * X * Y * Z          # 393216
REG_PLANES = B * C * (X - 1)           # 372 valid base planes
PLANES_PC = 48                         # plane slots per core (8*48 = 384)
UNITS_PC = PLANES_PC * (Y - 1)         # 1488 (plane, y) units per core
KU = 12                                # units per partition (128*12 = 1536)
FEM_P, FEM_F = 128, FEM_TOTAL // N_CORES // 128   # [128, 384] per core

_PROGRAM = None
_HOOK_PATCHED = False
# Bump whenever the BIR post-edit logic changes: the neuron compile cache
# keys on the HLO (which embeds the *unpatched* BIR), so a patch-logic change
# must perturb the program to force a recompile.
_BIR_REV = 4


def _fix_drain_waits(bir_json):
    """Walrus in this toolchain rejects instructions with >2 sync commands;
    Tile's kernel-tail drain waits on every proc used (no transitive
    reduction).  This kernel is a single dependency chain ending in the
    output DMA, whose completion implies every earlier wait, so the drain
    only needs that one semaphore."""
    import json

    j = json.loads(bir_json)
    for f in j.get("functions", []):
        last_dma_update = None
        for bb in f.get("blocks", []):
            for i in bb.get("instructions", []):
                if i.get("opcode") == "DMACopy":
                    ups = (i.get("sync_info") or {}).get("on_update") or []
                    if ups:
                        last_dma_update = ups[-1]
        if last_dma_update is None:
            continue
        for bb in f.get("blocks", []):
            for i in bb.get("instructions", []):
                if i.get("opcode") != "Drain":
                    continue
                si = i.get("sync_info") or {}
                waits = si.get("on_wait") or []
                if len(waits) + len(si.get("on_update") or []) <= 2:
                    continue
                keep = [w for w in waits if w.get("id") == last_dma_update.get("id")]
                assert keep, f"tail drain lacks final-DMA wait: {waits}"
                si["on_wait"] = keep
    return json.dumps(j).encode()


def _hoist_input_dmas(bir_json, input_names=("nf", "u")):
    """Move the input-load DMA triggers to the head of the first block so the
    HBM->SBUF transfers overlap the ~7.5us framework preamble instead of
    starting after it.  The triggers have no waits, their DMAHW semaphore
    updates don't interact with the barrier semaphores, and consumers keep
    their explicit waits, so ordering stays sound."""
    import json

    j = json.loads(bir_json)
    for f in j.get("functions", []):
        blocks = f.get("blocks", [])
        if not blocks:
            continue
        existing = {i.get("name") for bb in blocks for i in bb.get("instructions", [])}
        hoisted = []
        for bb in blocks:
            insts = bb.get("instructions", [])
            keep = []
            for i in insts:
                ins0 = (i.get("ins") or [{}])[0]
                if (i.get("opcode") == "DMACopy"
                        and not (i.get("sync_info") or {}).get("on_wait")
                        and ins0.get("memref") in input_names):
                    hoisted.append(i)
                else:
                    keep.append(i)
            bb["instructions"] = keep
        # Renumber so they sort before the barrier even if the backend orders
        # by instruction id rather than list position.
        for n, i in enumerate(hoisted):
            name = f"I-{n}"
            assert name not in existing, name
            i["name"] = name
            i["debug"] = 1
        blocks[0]["instructions"] = hoisted + blocks[0]["instructions"]
    return json.dumps(j).encode()


def _patch_compile_hook():
    global _HOOK_PATCHED
    if _HOOK_PATCHED:
        return
    import concourse.bass2jax as b2j

    orig = b2j.compile_bir_kernel

    def patched(bir_json, tmpdir, neff_name="file.neff"):
        return orig(_hoist_input_dmas(_fix_drain_waits(bir_json)),
                    tmpdir, neff_name=neff_name)

    b2j.compile_bir_kernel = patched
    _HOOK_PATCHED = True


def _build_program():
    import concourse.bass as bass
    import concourse.mybir as mybir
    from concourse import tile
    from contextlib import ExitStack

    f32 = mybir.dt.float32
    SUB = mybir.AluOpType.subtract
    MULT = mybir.AluOpType.mult

    nc = bass.Bass()
    nc.dram_tensor(f"patchrev{_BIR_REV}", [1, 1], f32)
    nf = nc.declare_dram_parameter("nf", [FEM_P, 2 * FEM_F], f32, isOutput=False)
    u = nc.declare_dram_parameter("u", [128, KU, 3, Z], f32, isOutput=False)
    out = nc.declare_dram_parameter("out", [128, 4], f32, isOutput=True)

    with tile.TileContext(nc) as tc, ExitStack() as ctx:
        pool = ctx.enter_context(tc.tile_pool(name="main", bufs=1))

        t_nf = pool.tile([FEM_P, 2 * FEM_F], f32)
        t_u = pool.tile([128, KU, 3, Z], f32)
        nc.sync.dma_start(out=t_nf[:], in_=nf[:, :])
        nc.sync.dma_start(out=t_u[:], in_=u[:, :, :, :])

        t_pack = pool.tile([128, 4], f32)

        # fem MSE partial: (net - fem)^2 row sums -> pack[:, 0]
        t_fd = pool.tile([FEM_P, FEM_F], f32)
        t_fsq = pool.tile([FEM_P, FEM_F], f32)
        nc.vector.tensor_tensor(out=t_fd[:], in0=t_nf[:, 0:FEM_F],
                                in1=t_nf[:, FEM_F:2 * FEM_F], op=SUB)
        nc.vector.scalar_tensor_tensor(out=t_fsq[:], in0=t_fd[:], scalar=1.0,
                                       in1=t_fd[:], op0=MULT, op1=MULT,
                                       accum_out=t_pack[0:FEM_P, 0:1])

        # regularization partials: unit row 0 = base, 1 = y+1 row, 2 = x+1 row
        base = t_u[:, :, 0, 0:31]
        shifts = [
            (t_u[:, :, 2, 0:31], 1),   # dx
            (t_u[:, :, 1, 0:31], 2),   # dy
            (t_u[:, :, 0, 1:32], 3),   # dz
        ]
        for shifted, col in shifts:
            t_d = pool.tile([128, KU, 31], f32, tag=f"d{col}")
            t_sq = pool.tile([128, KU, 31], f32, tag=f"sq{col}")
            nc.vector.tensor_tensor(out=t_d[:], in0=shifted, in1=base, op=SUB)
            nc.vector.scalar_tensor_tensor(out=t_sq[:], in0=t_d[:], scalar=1.0,
                                           in1=t_d[:], op0=MULT, op1=MULT,
                                           accum_out=t_pack[0:128, col:col + 1])

        nc.sync.dma_start(out=out[:, :], in_=t_pack[:])

    return nc


def _shard_inputs(network_mesh, fem_mesh, pred):
    predf = np.asarray(pred, dtype=np.float32).reshape(B * C, X, Y, Z)
    pad = N_CORES * PLANES_PC
    base_p = np.zeros((pad, Y, Z), np.float32)
    nxt_p = np.zeros((pad, Y, Z), np.float32)
    base_p[:REG_PLANES] = predf[:, : X - 1].reshape(REG_PLANES, Y, Z)
    nxt_p[:REG_PLANES] = predf[:, 1:].reshape(REG_PLANES, Y, Z)
    # [384, 31, 3, 32]: per (plane, y): base row, y+1 row, x+1-plane row
    u_all = np.stack(
        [base_p[:, : Y - 1], base_p[:, 1:], nxt_p[:, : Y - 1]], axis=2
    )
    netf = np.asarray(network_mesh, dtype=np.float32).reshape(N_CORES, FEM_P, FEM_F)
    femf = np.asarray(fem_mesh, dtype=np.float32).reshape(N_CORES, FEM_P, FEM_F)
    nf = np.concatenate([netf, femf], axis=2)  # [N_CORES, 128, 768]
    maps = []
    for c in range(N_CORES):
        uc = u_all[PLANES_PC * c : PLANES_PC * (c + 1)].reshape(UNITS_PC, 3, Z)
        up = np.zeros((128 * KU, 3, Z), np.float32)
        up[:UNITS_PC] = uc
        maps.append({
            "nf": np.ascontiguousarray(nf[c]),
            "u": up.reshape(128, KU, 3, Z),
        })
    return maps


def run_sharded(network_mesh, fem_mesh, pred, trace=False):
    """Compile+run on 8 cores; returns (loss_scalar, BassKernelResults)."""
    global _PROGRAM
    from concourse.bass_utils import run_bass_kernel_spmd

    _patch_compile_hook()
    if _PROGRAM is None:
        _PROGRAM = _build_program()
    in_maps = _shard_inputs(network_mesh, fem_mesh, pred)
    res = run_bass_kernel_spmd(_PROGRAM, in_maps, list(range(N_CORES)), trace=trace)
    fem_sum = 0.0
    reg_sum = 0.0
    for c in range(N_CORES):
        o = np.asarray(res.results[c]["out"], dtype=np.float64)
        fem_sum += o[:, 0].sum()
        reg_sum += o[:, 1:4].sum()
    loss = fem_sum / FEM_TOTAL + 0.1 * (reg_sum / (B * C))
    return np.asarray(loss, dtype=np.float32), res


def kernel(network_mesh, pc, fem_mesh, pred):
    loss, _ = run_sharded(network_mesh, fem_mesh, pred, trace=False)
    return loss


# revision 16
# speedup vs baseline: 1.6055x; 1.0318x over previous
"""Trainium2 Bass kernel for nn_MeshLoss.

The reference loss is:
    loss = mean((network_mesh - fem_mesh)^2)
         + 0.1 * sum_{dx,dy,dz} sum_spatial(mean_{B,C}(diff^2))
The chamfer/KNN block in the reference is dead code (its results are unused),
and `pc` does not influence the output, so the kernel computes only the two
reduction terms.

Sharding (8 cores): pred is viewed as 12*32 = 384 (bc, x) planes of [32, 32];
the 12*31 = 372 planes with x < 31 are regularization bases, 46-47 per core.
On the host each (plane, y<31) pair becomes a 3-row unit [base row, y+1 row,
x+1-plane row]; a core's 48*31 units (zero-padded to 1536) are laid out as
[128, 12, 3, 32], so every difference is an elementwise op over all 128
partitions with the y/z "::-1" bounds expressed as strided access patterns —
no masking, no invalid contributions.  network_mesh/fem_mesh are split into 8
chunks reshaped [128, 384] and packed side by side as [128, 768].

All compute runs on the Vector engine and there are only three DMAs; walrus
in this toolchain rejects instructions with more than 2 sync commands, so the
kernel must stay a single dependency chain (see _fix_drain_waits).
Each core emits per-partition partial sums [128, 4]; the host sums the 8
outputs and applies the 1/N and 0.1/12 weights.
"""

import numpy as np

B, C, X, Y, Z = 4, 3, 32, 32, 32
N_CORES = 8
FEM_TOTAL = B * C * X * Y * Z          # 393216
REG_PLANES = B * C * (X - 1)           # 372 valid base planes
PLANES_PC = 48                         # plane slots per core (8*48 = 384)
UNITS_PC = PLANES_PC * (Y - 1)         # 1488 (plane, y) units per core
KU = 12                                # units per partition (128*12 = 1536)
FEM_P, FEM_F = 128, FEM_TOTAL // N_CORES // 128   # [128, 384] per core

_PROGRAM = None
_HOOK_PATCHED = False
# Bump whenever the BIR post-edit logic changes: the neuron compile cache
# keys on the HLO (which embeds the *unpatched* BIR), so a patch-logic change
# must perturb the program to force a recompile.
_BIR_REV = 5


def _fix_drain_waits(bir_json):
    """Walrus in this toolchain rejects instructions with >2 sync commands;
    Tile's kernel-tail drain waits on every proc used (no transitive
    reduction).  This kernel is a single dependency chain ending in the
    output DMA, whose completion implies every earlier wait, so the drain
    only needs that one semaphore."""
    import json

    j = json.loads(bir_json)
    for f in j.get("functions", []):
        last_dma_update = None
        for bb in f.get("blocks", []):
            for i in bb.get("instructions", []):
                if i.get("opcode") == "DMACopy":
                    ups = (i.get("sync_info") or {}).get("on_update") or []
                    if ups:
                        last_dma_update = ups[-1]
        if last_dma_update is None:
            continue
        for bb in f.get("blocks", []):
            for i in bb.get("instructions", []):
                if i.get("opcode") != "Drain":
                    continue
                si = i.get("sync_info") or {}
                waits = si.get("on_wait") or []
                if len(waits) + len(si.get("on_update") or []) <= 2:
                    continue
                keep = [w for w in waits if w.get("id") == last_dma_update.get("id")]
                assert keep, f"tail drain lacks final-DMA wait: {waits}"
                si["on_wait"] = keep
    return json.dumps(j).encode()


def _hoist_input_dmas(bir_json, input_names=("nf", "u")):
    """Move the input-load DMA triggers to the head of the first block so the
    HBM->SBUF transfers overlap the ~7.5us framework preamble instead of
    starting after it.  The triggers have no waits, their DMAHW semaphore
    updates don't interact with the barrier semaphores, and consumers keep
    their explicit waits, so ordering stays sound."""
    import json

    j = json.loads(bir_json)
    for f in j.get("functions", []):
        blocks = f.get("blocks", [])
        if not blocks:
            continue
        existing = {i.get("name") for bb in blocks for i in bb.get("instructions", [])}
        hoisted = []
        for bb in blocks:
            insts = bb.get("instructions", [])
            keep = []
            for i in insts:
                ins0 = (i.get("ins") or [{}])[0]
                if (i.get("opcode") == "DMACopy"
                        and not (i.get("sync_info") or {}).get("on_wait")
                        and ins0.get("memref") in input_names):
                    hoisted.append(i)
                else:
                    keep.append(i)
            bb["instructions"] = keep
        # Renumber so they sort before the barrier even if the backend orders
        # by instruction id rather than list position.
        for n, i in enumerate(hoisted):
            name = f"I-{n}"
            assert name not in existing, name
            i["name"] = name
            i["debug"] = 1
        blocks[0]["instructions"] = hoisted + blocks[0]["instructions"]
    return json.dumps(j).encode()


def _patch_compile_hook():
    global _HOOK_PATCHED
    if _HOOK_PATCHED:
        return
    import concourse.bass2jax as b2j

    orig = b2j.compile_bir_kernel

    def patched(bir_json, tmpdir, neff_name="file.neff"):
        return orig(_hoist_input_dmas(_fix_drain_waits(bir_json)),
                    tmpdir, neff_name=neff_name)

    b2j.compile_bir_kernel = patched
    _HOOK_PATCHED = True


def _build_program():
    import concourse.bass as bass
    import concourse.mybir as mybir
    from concourse import tile
    from contextlib import ExitStack

    f32 = mybir.dt.float32
    SUB = mybir.AluOpType.subtract
    MULT = mybir.AluOpType.mult

    nc = bass.Bass()
    nc.dram_tensor(f"patchrev{_BIR_REV}", [1, 1], f32)
    nf = nc.declare_dram_parameter("nf", [FEM_P, 2 * FEM_F], f32, isOutput=False)
    u = nc.declare_dram_parameter("u", [128, KU, 3, Z], f32, isOutput=False)
    out = nc.declare_dram_parameter("out", [128, 4], f32, isOutput=True)

    with tile.TileContext(nc) as tc, ExitStack() as ctx:
        pool = ctx.enter_context(tc.tile_pool(name="main", bufs=1))

        t_nf = pool.tile([FEM_P, 2 * FEM_F], f32)
        t_u = pool.tile([128, KU, 3, Z], f32)
        # Separate HWDGE rings (SP and ACT) so the two loads transfer in
        # parallel instead of serializing on one ring.
        nc.sync.dma_start(out=t_nf[:], in_=nf[:, :])
        nc.scalar.dma_start(out=t_u[:], in_=u[:, :, :, :])

        t_pack = pool.tile([128, 4], f32)

        # fem MSE partial: (net - fem)^2 row sums -> pack[:, 0]
        t_fd = pool.tile([FEM_P, FEM_F], f32)
        t_fsq = pool.tile([FEM_P, FEM_F], f32)
        nc.vector.tensor_tensor(out=t_fd[:], in0=t_nf[:, 0:FEM_F],
                                in1=t_nf[:, FEM_F:2 * FEM_F], op=SUB)
        nc.vector.scalar_tensor_tensor(out=t_fsq[:], in0=t_fd[:], scalar=1.0,
                                       in1=t_fd[:], op0=MULT, op1=MULT,
                                       accum_out=t_pack[0:FEM_P, 0:1])

        # regularization partials: unit row 0 = base, 1 = y+1 row, 2 = x+1 row
        base = t_u[:, :, 0, 0:31]
        shifts = [
            (t_u[:, :, 2, 0:31], 1),   # dx
            (t_u[:, :, 1, 0:31], 2),   # dy
            (t_u[:, :, 0, 1:32], 3),   # dz
        ]
        for shifted, col in shifts:
            t_d = pool.tile([128, KU, 31], f32, tag=f"d{col}")
            t_sq = pool.tile([128, KU, 31], f32, tag=f"sq{col}")
            nc.vector.tensor_tensor(out=t_d[:], in0=shifted, in1=base, op=SUB)
            nc.vector.scalar_tensor_tensor(out=t_sq[:], in0=t_d[:], scalar=1.0,
                                           in1=t_d[:], op0=MULT, op1=MULT,
                                           accum_out=t_pack[0:128, col:col + 1])

        nc.sync.dma_start(out=out[:, :], in_=t_pack[:])

    return nc


def _shard_inputs(network_mesh, fem_mesh, pred):
    predf = np.asarray(pred, dtype=np.float32).reshape(B * C, X, Y, Z)
    pad = N_CORES * PLANES_PC
    base_p = np.zeros((pad, Y, Z), np.float32)
    nxt_p = np.zeros((pad, Y, Z), np.float32)
    base_p[:REG_PLANES] = predf[:, : X - 1].reshape(REG_PLANES, Y, Z)
    nxt_p[:REG_PLANES] = predf[:, 1:].reshape(REG_PLANES, Y, Z)
    # [384, 31, 3, 32]: per (plane, y): base row, y+1 row, x+1-plane row
    u_all = np.stack(
        [base_p[:, : Y - 1], base_p[:, 1:], nxt_p[:, : Y - 1]], axis=2
    )
    netf = np.asarray(network_mesh, dtype=np.float32).reshape(N_CORES, FEM_P, FEM_F)
    femf = np.asarray(fem_mesh, dtype=np.float32).reshape(N_CORES, FEM_P, FEM_F)
    nf = np.concatenate([netf, femf], axis=2)  # [N_CORES, 128, 768]
    maps = []
    for c in range(N_CORES):
        uc = u_all[PLANES_PC * c : PLANES_PC * (c + 1)].reshape(UNITS_PC, 3, Z)
        up = np.zeros((128 * KU, 3, Z), np.float32)
        up[:UNITS_PC] = uc
        maps.append({
            "nf": np.ascontiguousarray(nf[c]),
            "u": up.reshape(128, KU, 3, Z),
        })
    return maps


def run_sharded(network_mesh, fem_mesh, pred, trace=False):
    """Compile+run on 8 cores; returns (loss_scalar, BassKernelResults)."""
    global _PROGRAM
    from concourse.bass_utils import run_bass_kernel_spmd

    _patch_compile_hook()
    if _PROGRAM is None:
        _PROGRAM = _build_program()
    in_maps = _shard_inputs(network_mesh, fem_mesh, pred)
    res = run_bass_kernel_spmd(_PROGRAM, in_maps, list(range(N_CORES)), trace=trace)
    fem_sum = 0.0
    reg_sum = 0.0
    for c in range(N_CORES):
        o = np.asarray(res.results[c]["out"], dtype=np.float64)
        fem_sum += o[:, 0].sum()
        reg_sum += o[:, 1:4].sum()
    loss = fem_sum / FEM_TOTAL + 0.1 * (reg_sum / (B * C))
    return np.asarray(loss, dtype=np.float32), res


def kernel(network_mesh, pc, fem_mesh, pred):
    loss, _ = run_sharded(network_mesh, fem_mesh, pred, trace=False)
    return loss
